# revision 5
# baseline (speedup 1.0000x reference)
"""Trainium2 Bass kernel for a custom LSTM.

Problem: x[64,512,1024] fp32, W[1024,4096], U[1024,4096], bias[4096].
  xW = einsum('bsi,ig->sbg', x, W) + bias            (precompute, "phase 1")
  then a 512-step LSTM recurrence over S with h@U      ("phase 2")
Returns (hidden_seq[64,512,1024], (h_T[64,1024], c_T[64,1024])).

Strategy: pure data-parallel over batch (8 batches per core, 8 cores, no
collectives).  Everything on-chip lives in a hidden-on-partitions layout so
the per-step elementwise work uses all 128 lanes:

  - gate columns of W/U are permuted host-side into chunks c = t*8+k
    (t in {i,f,o,g}, k = hidden-128-block), each 128 wide, and within a
    chunk split into four 32-wide col-groups j for PE column tiling.
  - per step, gates^T[128p, (c,b)] = U^T @ h^T accumulates in PSUM via
    1024 small matmuls (stationary U tile [128,32], moving h^T [128,8]),
    4 concurrent via tile_position col-groups.
  - elementwise (sigmoid/tanh/cell update) operates on [128, 32, 8] tiles.
  - h state is produced directly in h^T layout -> no transposes anywhere.
"""

import os
import numpy as np

B, S, I, H = 64, 512, 1024, 1024
NCORES = 8
BL = B // NCORES          # 8 batches per core
NK = H // 128             # 8 hidden 128-blocks
NC_CH = 32                # gate chunks (4 types x 8 blocks)
NTOK = S * BL             # 4096 tokens per core
NBLK = 32                 # xw blocks of 16 steps
STEPS_PER_BLK = 16
BODY_STEPS = 32           # steps per For_i body (2 xw blocks)
N_BODIES = S // BODY_STEPS  # 16

LAST_EXEC_NS = None


def _build_nc():
    import concourse.bass as bass
    import concourse.tile as tile
    from concourse import bacc, mybir

    f32 = mybir.dt.float32
    AF = mybir.ActivationFunctionType
    ds = bass.ds

    nc = bacc.Bacc("TRN2", target_bir_lowering=False, debug=False,
                   num_devices=NCORES)

    xT = nc.declare_dram_parameter("xT", [128, NK, NTOK], f32, isOutput=False)
    Wp = nc.declare_dram_parameter("Wp", [128, NK, NC_CH, 128], f32, isOutput=False)
    Up = nc.declare_dram_parameter("Up", [128, NK, NC_CH, 4, 32], f32, isOutput=False)
    biasp = nc.declare_dram_parameter("biasp", [128, NC_CH], f32, isOutput=False)
    hseq = nc.declare_dram_parameter("hseq", [N_BODIES, BODY_STEPS, 128, 64], f32,
                                     isOutput=True)
    cout = nc.declare_dram_parameter("cout", [128, 64], f32, isOutput=True)

    # xw scratch: [blk, chunk, p, 16 steps * 8 batch]; +1 dummy blk for the
    # tail prefetch.
    xw = nc.dram_tensor("xw_scratch", [NBLK + 1, 128, NC_CH, 128], f32)

    with tile.TileContext(nc) as tc:
        # ---------------- phase 1: xw[s] = x_t @ W + bias ----------------
        with tc.tile_pool(name="p1w", bufs=1) as p1w, \
             tc.tile_pool(name="p1x", bufs=2) as p1x, \
             tc.tile_pool(name="p1o", bufs=4) as p1o, \
             tc.tile_pool(name="p1b", bufs=1) as p1b, \
             tc.tile_pool(name="p1ps", bufs=4, space="PSUM") as p1ps:
            w_sb = p1w.tile([128, NK, NC_CH, 128], f32)
            nc.sync.dma_start(w_sb[:], Wp[:])
            bias_sb = p1b.tile([128, NC_CH], f32)
            nc.sync.dma_start(bias_sb[:], biasp[:])

            for n in range(8):            # 8 token tiles of 512
                x_sb = p1x.tile([128, NK, 512], f32)
                nc.sync.dma_start(x_sb[:], xT[:, :, n * 512:(n + 1) * 512])
                for c in range(NC_CH):
                    ps = p1ps.tile([128, 512], f32)
                    for ki in range(NK):
                        nc.tensor.matmul(ps[:], w_sb[:, ki, c, :],
                                         x_sb[:, ki, :],
                                         start=(ki == 0), stop=(ki == NK - 1))
                    o_sb = p1o.tile([128, 512], f32)
                    nc.scalar.add(o_sb[:], ps[:], bias_sb[:, c:c + 1])
                    for q in range(4):
                        nc.sync.dma_start(xw[4 * n + q, :, c, :],
                                          o_sb[:, q * 128:(q + 1) * 128])

        # ---------------- phase 2: the recurrence ----------------
        with tc.tile_pool(name="p2u", bufs=1) as p2u, \
             tc.tile_pool(name="p2xw", bufs=1) as p2xw, \
             tc.tile_pool(name="p2st", bufs=1) as p2st, \
             tc.tile_pool(name="p2g", bufs=2) as p2g, \
             tc.tile_pool(name="p2t", bufs=3) as p2t, \
             tc.tile_pool(name="p2ps", bufs=2, space="PSUM") as p2ps:
            u_sb = p2u.tile([128, NK, NC_CH, 4, 32], f32)
            nc.sync.dma_start(u_sb[:], Up[:])

            h_st = [p2st.tile([128, NK, BL], f32, tag=f"h{i}", name=f"h{i}") for i in range(2)]
            c_st = [p2st.tile([128, NK, BL], f32, tag=f"c{i}", name=f"c{i}") for i in range(2)]
            nc.gpsimd.memset(h_st[0][:], 0.0)
            nc.gpsimd.memset(c_st[0][:], 0.0)

            xwb = [p2xw.tile([128, NC_CH, 128], f32, tag=f"xwb{i}", name=f"xwb{i}") for i in range(2)]
            nc.sync.dma_start(xwb[0][:], xw[0])

            def step(hseq_blk, ls, xw_tile, xw_ls):
                h_in = h_st[ls % 2]
                h_out = h_st[(ls + 1) % 2]
                c_in = c_st[ls % 2]
                c_out = c_st[(ls + 1) % 2]

                ps = p2ps.tile([128, NC_CH, BL], f32)
                for c in range(NC_CH):
                    for ki in range(NK):
                        for j in range(4):
                            nc.tensor.matmul(
                                ps[32 * j:32 * (j + 1), c, :],
                                u_sb[:, ki, c, j, :],
                                h_in[:, ki, :],
                                start=(ki == 0), stop=(ki == NK - 1),
                                tile_position=(0, 32 * j),
                            )
                gates = p2g.tile([128, NC_CH, BL], f32)
                nc.vector.tensor_add(gates[:], ps[:],
                                     xw_tile[:, :, xw_ls * 8:(xw_ls + 1) * 8])
                act = p2g.tile([128, NC_CH, BL], f32, tag="act")
                nc.scalar.activation(act[:, 0:24, :], gates[:, 0:24, :], AF.Sigmoid)
                nc.scalar.activation(act[:, 24:32, :], gates[:, 24:32, :], AF.Tanh)
                m1 = p2t.tile([128, NK, BL], f32, tag="m1")
                nc.vector.tensor_mul(m1[:], act[:, 0:8, :], act[:, 24:32, :])
                nc.vector.tensor_mul(c_out[:], act[:, 8:16, :], c_in[:])
                nc.vector.tensor_add(c_out[:], c_out[:], m1[:])
                tc_t = p2t.tile([128, NK, BL], f32, tag="tc")
                nc.scalar.activation(tc_t[:], c_out[:], AF.Tanh)
                nc.vector.tensor_mul(h_out[:], act[:, 16:24, :], tc_t[:])
                nc.sync.dma_start(hseq_blk[0, ls], h_out[:])

            with tc.For_i(0, N_BODIES, 1) as bi:
                hseq_blk = hseq[ds(bi, 1)]
                # prefetch odd block while steps 0..15 run on even block
                nc.sync.dma_start(xwb[1][:], xw[ds(2 * bi + 1, 1)][0])
                for ls in range(STEPS_PER_BLK):
                    step(hseq_blk, ls, xwb[0], ls)
                # prefetch next even block (dummy blk 32 on the last body)
                nc.sync.dma_start(xwb[0][:], xw[ds(2 * bi + 2, 1)][0])
                for ls in range(STEPS_PER_BLK):
                    step(hseq_blk, STEPS_PER_BLK + ls, xwb[1], ls)

            nc.sync.dma_start(cout[:], c_st[0][:])

    nc.finalize()
    return nc


# host-side permutation of gate columns: chunk c = t*8+k, col-group j, w
_TORIG = np.array([0, 1, 3, 2])  # our type order (i,f,o,g) -> reference (i,f,g,o)


def _col_perm():
    cc, jj, ww = np.meshgrid(np.arange(NC_CH), np.arange(4), np.arange(32),
                             indexing="ij")
    return (1024 * _TORIG[cc >> 3] + 128 * (cc & 7) + 32 * jj + ww).reshape(-1)


def _prepare_in_maps(x, W, U, bias):
    x = np.ascontiguousarray(np.asarray(x, dtype=np.float32))
    W = np.asarray(W, dtype=np.float32)
    U = np.asarray(U, dtype=np.float32)
    bias = np.asarray(bias, dtype=np.float32)

    perm = _col_perm()
    Wp = np.ascontiguousarray(
        W[:, perm].reshape(NK, 128, 4096).transpose(1, 0, 2)
        .reshape(128, NK, NC_CH, 128))
    Up = np.ascontiguousarray(
        U[:, perm].reshape(NK, 128, 4096).transpose(1, 0, 2)
        .reshape(128, NK, NC_CH, 4, 32))
    biasp = np.ascontiguousarray(bias[perm].reshape(NC_CH, 128).T)

    in_maps = []
    for cid in range(NCORES):
        xs = x[BL * cid:BL * (cid + 1)]                      # [8, 512, 1024]
        xT = np.ascontiguousarray(
            xs.transpose(2, 1, 0).reshape(NK, 128, NTOK).transpose(1, 0, 2))
        in_maps.append({"xT": xT, "Wp": Wp, "Up": Up, "biasp": biasp})
    return in_maps


def _assemble(results):
    hidden_seq = np.empty((B, S, H), dtype=np.float32)
    c_T = np.empty((B, H), dtype=np.float32)
    for cid in range(NCORES):
        hs = results[cid]["hseq"].reshape(S, 128, NK, BL)
        hidden_seq[BL * cid:BL * (cid + 1)] = (
            hs.transpose(3, 0, 2, 1).reshape(BL, S, H))
        co = results[cid]["cout"].reshape(128, NK, BL)
        c_T[BL * cid:BL * (cid + 1)] = co.transpose(2, 1, 0).reshape(BL, H)
    h_T = np.ascontiguousarray(hidden_seq[:, -1, :])
    return hidden_seq, (h_T, c_T)


def kernel(x, W, U, bias):
    global LAST_EXEC_NS
    from concourse.bass_utils import run_bass_kernel_spmd

    in_maps = _prepare_in_maps(x, W, U, bias)
    nc = _build_nc()
    trace = bool(int(os.environ.get("KERNEL_TRACE", "0")))
    res = run_bass_kernel_spmd(nc, in_maps, list(range(NCORES)), trace=trace)
    LAST_EXEC_NS = res.exec_time_ns
    return _assemble(res.results)


if __name__ == "__main__":
    rng = np.random.default_rng(0)
    stdv = 1.0 / np.sqrt(H)
    x = rng.standard_normal((B, S, I), dtype=np.float32)
    W = rng.uniform(-stdv, stdv, (I, 4 * H)).astype(np.float32)
    U = rng.uniform(-stdv, stdv, (H, 4 * H)).astype(np.float32)
    bias = rng.uniform(-stdv, stdv, (4 * H,)).astype(np.float32)
    out = kernel(x=x, W=W, U=U, bias=bias)
    print("ran", out[0].shape, out[1][0].shape, out[1][1].shape)
